# revision 6
# baseline (speedup 1.0000x reference)
"""v3 Trainium2 Bass kernel for nn_CustomWeightedTensorProduct.

vs v2: host interleaves path-2/path-4 weights so one mult op per i covers
both paths; groups 01+24 share one product buffer and ONE fused 4-level
add-tree (in-place halving, no scratch tiles); group 3 reuses the buffer.
DVE op count per tile drops ~40 -> ~26.

w layout (per edge, int8, u-fastest):
  [p0 (w,u) 256 | p1 256 | p2&p4 as (k,w,u) k={p2,p4} 512 | p3 256]
"""

import sys

if "/opt/trn_rl_repo" not in sys.path:
    sys.path.insert(0, "/opt/trn_rl_repo")

import numpy as np

Z_FULL = 100000
N_CORES = 8
P = 128
C = 14
TILE_E = P * C              # 1792
N_TILES = 7
ZC = TILE_E * N_TILES       # 12544
Z_PAD = ZC * N_CORES        # 100352

SQRT2 = 2.0 ** 0.5
SQRT3 = 3.0 ** 0.5
K0 = 1.0 / (32.0 ** 0.5)
K1 = 1.0 / (48.0 ** 0.5)
K3_OVER_K1 = (K0 / SQRT3) / K1      # = 1/sqrt(2)
KD = K1 / SQRT2

X1W = 112                   # 16 + 48 + 48
X2W = 8


def build_bass(n_tiles=N_TILES, reps=1):
    import contextlib
    import concourse.bass as bass  # noqa: F401
    import concourse.bacc as bacc
    import concourse.mybir as mybir
    from concourse.tile import TileContext

    zc = TILE_E * n_tiles
    f32 = mybir.dt.float32
    bf = mybir.dt.bfloat16
    i8 = getattr(mybir.dt, "int8", None) or mybir.dt.uint8
    ADD = mybir.AluOpType.add
    MUL = mybir.AluOpType.mult
    SUB = mybir.AluOpType.subtract
    AX = mybir.AxisListType.X

    # HBM layout = SBUF layout: row r = (tile*128 + partition), holding that
    # partition's C edges contiguously -> every DMA is a plain [128, C*D]
    # row-block copy (big bursts, no per-edge segments).
    nc = bacc.Bacc(None, target_bir_lowering=False)
    nrow = n_tiles * P
    x1_d = nc.dram_tensor("x1", [nrow, C * X1W], bf, kind="ExternalInput")
    x2_d = nc.dram_tensor("x2", [nrow, C * X2W], bf, kind="ExternalInput")
    w_d = nc.dram_tensor("w", [nrow, C * 1280], i8, kind="ExternalInput")
    out_d = nc.dram_tensor("out", [nrow, C * 64], bf, kind="ExternalOutput")

    with TileContext(nc) as tc:
        with (
            tc.tile_pool(name="io", bufs=2) as pio,
            tc.tile_pool(name="wb", bufs=1) as pw,
            tc.tile_pool(name="prod", bufs=1) as pp,
            tc.tile_pool(name="small", bufs=2) as ps,
            tc.For_i(0, reps) if reps > 1 else contextlib.nullcontext(),
        ):
            for t in range(n_tiles):
                r0 = t * P
                wv = w_d[r0:r0 + P, :]
                x1v = x1_d[r0:r0 + P, :]
                x2v = x2_d[r0:r0 + P, :]
                outv = out_d[r0:r0 + P, :]

                # ---- loads ----
                W8 = pio.tile([P, C * 1280], i8)
                X1t = pio.tile([P, C * X1W], bf)
                X2t = pio.tile([P, C * X2W], bf)
                OUTt = pio.tile([P, C * 64], bf)
                nc.sync.dma_start(X1t[:], x1v[:, :])
                nc.sync.dma_start(X2t[:], x2v[:, :])
                nc.sync.dma_start(W8[:], wv[:, :])

                x1b = X1t.rearrange("p (c d) -> p c d", d=X1W)
                s10 = x1b[:, :, 0:16]                    # (c, u)
                s11T = x1b[:, :, 16:64].rearrange(
                    "p c (i u) -> p c i u", i=3)         # (c, i, u)
                s11 = x1b[:, :, 64:112].rearrange(
                    "p c (u i) -> p c u i", i=3)         # (c, u, i)
                x2b = X2t.rearrange("p (c d) -> p c d", d=X2W)
                s20K0 = x2b[:, :, 0]                     # (c,)
                s21K1 = x2b[:, :, 1:4]                   # (c, 3)
                s20K1 = x2b[:, :, 4]
                s21KD = x2b[:, :, 5:8]

                w8 = W8.rearrange("p (c q) -> p c q", q=1280)

                Wa = pw.tile([P, C * 512], bf)   # W01
                Wb = pw.tile([P, C * 512], bf)   # W24
                Wc = pw.tile([P, C * 256], bf)   # W3
                PR = pp.tile([P, C * 2304], bf)  # products g01|g24|g3
                prv = PR.rearrange("p (c x) -> p c x", x=2304)

                def cast_w(dst, lo, hi):
                    n = hi - lo
                    t_ = dst.rearrange(
                        "p (c x) -> p c x", x=n if n != 512 else 512)
                    t_ = t_[:, :, 0:n] if n == 512 else t_
                    nc.scalar.copy(t_, w8[:, :, lo:hi])
                    return t_

                def tree_inplace(kn, view, ps_out):
                    """in-place halving reduce over innermost 16 of
                    view [P, C, kn, 16]; result -> ps_out [P, C*kn]."""
                    nc.vector.tensor_tensor(
                        view[:, :, :, 8:16], view[:, :, :, 0:8],
                        view[:, :, :, 8:16], ADD)
                    nc.vector.tensor_tensor(
                        view[:, :, :, 12:16], view[:, :, :, 8:12],
                        view[:, :, :, 12:16], ADD)
                    nc.vector.tensor_tensor(
                        view[:, :, :, 14:16], view[:, :, :, 12:14],
                        view[:, :, :, 14:16], ADD)
                    vo = ps_out.rearrange("p (c k) -> p c k", k=kn)
                    nc.vector.tensor_tensor(
                        vo, view[:, :, :, 14], view[:, :, :, 15], ADD)
                    return ps_out

                # ---- b[u] = <s11[u,:], s21K1> (early: only needs x1/x2) ----
                Bp = ps.tile([P, C * 48], bf)
                bpv = Bp.rearrange("p (c u i) -> p c u i", u=16, i=3)
                nc.vector.tensor_tensor(
                    bpv, s11,
                    s21K1.unsqueeze(2).broadcast_to([P, C, 16, 3]), MUL)
                bT32 = ps.tile([P, C * 16], f32)
                btv32 = bT32.rearrange("p (c u) -> p c u", u=16)
                nc.vector.tensor_reduce(btv32, bpv, axis=AX, op=ADD)
                bT = ps.tile([P, C * 16], bf)
                btv = bT.rearrange("p (c u) -> p c u", u=16)
                nc.scalar.copy(bT[:], bT32[:])

                # ---- mults: g01 (1 op) + g24 (3 ops) ----
                W01 = cast_w(Wa, 0, 512)
                nc.vector.tensor_tensor(
                    prv[:, :, 0:512].rearrange("p c (k u) -> p c k u", u=16),
                    W01.rearrange("p c (k u) -> p c k u", u=16),
                    s10.unsqueeze(2).broadcast_to([P, C, 32, 16]), MUL)
                W24 = cast_w(Wb, 512, 1024)
                w24v = W24.rearrange("p c (k u) -> p c k u", u=16)  # k=32
                for i in range(3):
                    nc.vector.tensor_tensor(
                        prv[:, :, 512 + i * 512: 512 + (i + 1) * 512]
                        .rearrange("p c (k u) -> p c k u", u=16),
                        w24v,
                        s11T[:, :, i, :].unsqueeze(2).broadcast_to(
                            [P, C, 32, 16]), MUL)

                # ---- group 3 mult (b is ready early) ----
                W3 = cast_w(Wc, 1024, 1280)
                p3 = prv[:, :, 2048:2304].rearrange(
                    "p c (w u) -> p c w u", u=16)
                nc.vector.tensor_tensor(
                    p3, W3.rearrange("p c (w u) -> p c w u", u=16),
                    btv.unsqueeze(2).broadcast_to([P, C, 16, 16]), MUL)

                # ---- ONE fused tree over g01+g24+g3 (k = 144 blocks) ----
                T = ps.tile([P, C * 144], bf)
                tree_inplace(
                    144, PR.rearrange("p (c k u) -> p c k u", k=144, u=16), T)
                tv = T.rearrange("p (c k) -> p c k", k=144)
                T01v = tv[:, :, 0:32].rearrange("p c (g w) -> p c g w", g=2)
                # g24 block order: [i(3), k2(2), w(16)]
                t24 = tv[:, :, 32:128].rearrange(
                    "p c (i k2 w) -> p c i k2 w", i=3, k2=2)
                T2v = t24[:, :, :, 0, :]                 # (c, i, w)
                T4v = t24[:, :, :, 1, :]
                t3v = tv[:, :, 128:144]

                outc = OUTt.rearrange("p (c d) -> p c d", d=64)

                # ---- out0 = T01[0]*s20K0 + t3*(K3/K1) ----
                o0a = ps.tile([P, C * 16], bf)
                o0av = o0a.rearrange("p (c w) -> p c w", w=16)
                nc.vector.tensor_tensor(
                    o0av, T01v[:, :, 0, :],
                    s20K0.unsqueeze(2).broadcast_to([P, C, 16]), MUL)
                nc.vector.scalar_tensor_tensor(
                    outc[:, :, 0:16], t3v, float(K3_OVER_K1), o0av, MUL, ADD)

                # ---- out1 assembly (DVE) ----
                o1a = ps.tile([P, C * 48], bf)
                o1av = o1a.rearrange("p (c i w) -> p c i w", i=3, w=16)
                nc.vector.tensor_tensor(
                    o1av,
                    T01v[:, :, 1, :].unsqueeze(2).broadcast_to([P, C, 3, 16]),
                    s21K1.unsqueeze(3).broadcast_to([P, C, 3, 16]), MUL)
                o1b = ps.tile([P, C * 48], bf)
                o1bv = o1b.rearrange("p (c i w) -> p c i w", i=3, w=16)
                nc.vector.tensor_tensor(
                    o1bv, T2v,
                    s20K1.unsqueeze(2).unsqueeze(3).broadcast_to([P, C, 3, 16]),
                    MUL)
                o1s = ps.tile([P, C * 48], bf)
                o1sv = o1s.rearrange("p (c i w) -> p c i w", i=3, w=16)
                nc.vector.tensor_tensor(o1sv, o1av, o1bv, ADD)

                # cross(T4, s21KD): wrap-around slices, no T4 duplication
                X2D = ps.tile([P, C * 6], bf)
                x2dd = X2D.rearrange("p (c e) -> p c e", e=6)
                nc.vector.tensor_copy(
                    X2D.rearrange("p (c r k) -> p c r k", r=2, k=3),
                    s21KD.unsqueeze(2).broadcast_to([P, C, 2, 3]))
                m1 = ps.tile([P, C * 48], bf)
                m1v = m1.rearrange("p (c i w) -> p c i w", i=3, w=16)
                nc.vector.tensor_tensor(
                    m1v[:, :, 0:2, :], T4v[:, :, 1:3, :],
                    x2dd[:, :, 2:4].unsqueeze(3).broadcast_to([P, C, 2, 16]),
                    MUL)
                nc.vector.tensor_tensor(
                    m1v[:, :, 2:3, :], T4v[:, :, 0:1, :],
                    x2dd[:, :, 4:5].unsqueeze(3).broadcast_to([P, C, 1, 16]),
                    MUL)
                m2 = ps.tile([P, C * 48], bf)
                m2v = m2.rearrange("p (c i w) -> p c i w", i=3, w=16)
                nc.vector.tensor_tensor(
                    m2v[:, :, 0:1, :], T4v[:, :, 2:3, :],
                    x2dd[:, :, 1:2].unsqueeze(3).broadcast_to([P, C, 1, 16]),
                    MUL)
                nc.vector.tensor_tensor(
                    m2v[:, :, 1:3, :], T4v[:, :, 0:2, :],
                    x2dd[:, :, 2:4].unsqueeze(3).broadcast_to([P, C, 2, 16]),
                    MUL)
                crs = ps.tile([P, C * 48], bf)
                crsv = crs.rearrange("p (c i w) -> p c i w", i=3, w=16)
                nc.vector.tensor_tensor(crsv, m1v, m2v, SUB)

                out1ap = outc[:, :, 16:64].rearrange("p c (w i) -> p c i w", i=3)
                nc.vector.tensor_tensor(out1ap, o1sv, crsv, ADD)

                # ---- store ----
                nc.sync.dma_start(outv[:, :], OUTt[:])

    nc.compile()
    return nc


_CACHE = {}
TRACE = False
LAST_RESULTS = None


def _get_nc():
    if "nc" not in _CACHE:
        _CACHE["nc"] = build_bass()
    return _CACHE["nc"]


def _host_prep(x1, x2, w):
    import ml_dtypes
    bfnp = ml_dtypes.bfloat16
    x1 = np.asarray(x1, dtype=np.float32)
    x2 = np.asarray(x2, dtype=np.float32)
    w = np.asarray(w, dtype=np.float32)
    z = x1.shape[0]

    s = np.abs(w).max(axis=1, keepdims=True) / 127.0       # (z, 1)
    s_safe = np.maximum(s, 1e-30)
    wq = np.clip(np.round(w / s_safe), -127, 127).astype(np.int8)
    # [path, u, w] -> [path, w, u]; then interleave paths 2,4 as (k,w,u)
    wq = wq.reshape(z, 5, 16, 16).transpose(0, 1, 3, 2)    # (z, 5, w, u)
    w24 = np.stack([wq[:, 2], wq[:, 4]], axis=1)           # (z, 2, w, u)
    wq = np.concatenate([
        wq[:, 0].reshape(z, 256), wq[:, 1].reshape(z, 256),
        w24.reshape(z, 512), wq[:, 3].reshape(z, 256)], axis=1)
    wq = np.ascontiguousarray(wq)

    s10 = x1[:, :16] * s
    s11 = x1[:, 16:64].reshape(z, 16, 3) * s[:, :, None]
    s11T = np.ascontiguousarray(s11.transpose(0, 2, 1))
    x1p = np.concatenate(
        [s10, s11T.reshape(z, 48), s11.reshape(z, 48)], axis=1).astype(bfnp)

    s20 = x2[:, 0:1]
    s21 = x2[:, 1:4]
    x2p = np.concatenate(
        [s20 * K0, s21 * K1, s20 * K1, s21 * KD], axis=1).astype(bfnp)
    return x1p, x2p, wq


def _to_rows(a):
    """[Z_PAD, D] edge-major -> [8*7*128, C*D] partition-row-major."""
    d = a.shape[1]
    return np.ascontiguousarray(
        a.reshape(N_CORES, N_TILES, P, C, d).reshape(
            N_CORES * N_TILES * P, C * d))


def _from_rows(a):
    """[8*7*128, C*64] -> [Z_PAD, 64]."""
    return np.ascontiguousarray(
        a.reshape(N_CORES, N_TILES, P, C, 64).reshape(Z_PAD, 64))


def gather_out(raw):
    return _from_rows(raw)


def prep_global(inputs):
    """bench2 hook: full padded global input arrays (row layout)."""
    x1p, x2p, wq = _host_prep(inputs["x1"], inputs["x2"], inputs["w"])
    z = x1p.shape[0]
    pad = Z_PAD - z
    return {
        "x1": _to_rows(np.pad(x1p, ((0, pad), (0, 0)))),
        "x2": _to_rows(np.pad(x2p, ((0, pad), (0, 0)))),
        "w": _to_rows(np.pad(wq, ((0, pad), (0, 0)))),
    }


def kernel(x1, x2, w):
    global LAST_RESULTS
    from concourse.bass_utils import run_bass_kernel_spmd

    z = np.asarray(x1).shape[0]
    x1p, x2p, wq = _host_prep(x1, x2, w)
    pad = Z_PAD - z
    x1r = _to_rows(np.pad(x1p, ((0, pad), (0, 0))))
    x2r = _to_rows(np.pad(x2p, ((0, pad), (0, 0))))
    wr = _to_rows(np.pad(wq, ((0, pad), (0, 0))))

    rows = N_TILES * P
    in_maps = []
    for k in range(N_CORES):
        sl = slice(k * rows, (k + 1) * rows)
        in_maps.append({
            "x1": np.ascontiguousarray(x1r[sl]),
            "x2": np.ascontiguousarray(x2r[sl]),
            "w": np.ascontiguousarray(wr[sl]),
        })

    nc = _get_nc()
    res = run_bass_kernel_spmd(
        nc, in_maps, core_ids=list(range(N_CORES)), trace=TRACE)
    LAST_RESULTS = res
    out = np.concatenate([np.asarray(r["out"]) for r in res.results], axis=0)
    out = _from_rows(out)
    return np.ascontiguousarray(out[:z].astype(np.float32))
